# revision 4
# baseline (speedup 1.0000x reference)
"""MoE (top-2 of 8 experts) Trainium2 kernel.

Strategy: expert-parallel across the 8 NeuronCores. The router
(x @ gate_w -> softmax -> top-2 -> renormalize) is tiny (0.05% of the
FLOPs) and data-dependent, so it runs on the host in float64; the
host then gathers each expert's tokens into a fixed-capacity buffer
and ships one expert per core. Each core computes the dense FFN
    yT = w2.T @ gelu(w1.T @ xT)
for its expert over its gathered tokens (token dim on the matmul free
axis, so no on-device transposes are needed), and the host scatter-adds
the combine-weighted results back into the full output.

Compute is bf16 (weights SBUF-resident, fp32 PSUM accumulation).
"""

import numpy as np
import ml_dtypes

import concourse.bacc as bacc
import concourse.tile as tile
import concourse.mybir as mybir
from concourse.bass_utils import run_bass_kernel_spmd

NUM_EXPERTS = 8
TOP_K = 2
C = 1024  # model dim
F = 4096  # ffn dim
N_CORES = 8
TOK_TILE = 512  # matmul free-dim tile (one fp32 PSUM bank)

_cache: dict = {}


def _build(cap: int, loop_iters: int = 1):
    """Build the per-core FFN module for `cap` gathered tokens.

    loop_iters > 1 wraps the token-tile compute in a hardware For loop
    (weights stay resident) — used only for benchmarking.
    """
    bf16 = mybir.dt.bfloat16
    f32 = mybir.dt.float32
    CT = C // 128  # 8 partition tiles over model dim
    FT = F // 128  # 32 partition tiles over ffn dim

    tok_tiles = []
    t0 = 0
    while t0 < cap:
        tok_tiles.append((t0, min(TOK_TILE, cap - t0)))
        t0 += TOK_TILE

    nc = bacc.Bacc("TRN2", target_bir_lowering=False, debug=False)
    xT_d = nc.dram_tensor("xT", [C, cap], bf16, kind="ExternalInput")
    w1_d = nc.dram_tensor("w1", [C, F], bf16, kind="ExternalInput")
    w2_d = nc.dram_tensor("w2", [F, C], bf16, kind="ExternalInput")
    yT_d = nc.dram_tensor("yT", [C, cap], f32, kind="ExternalOutput")

    with tile.TileContext(nc) as tc:
        with (
            tc.tile_pool(name="wpool", bufs=1) as wpool,
            tc.tile_pool(name="xpool", bufs=2 * CT) as xpool,
            tc.tile_pool(name="hpool", bufs=FT + 2) as hpool,
            tc.tile_pool(name="ypool", bufs=4) as ypool,
            tc.tile_pool(name="ps1", bufs=2, space="PSUM") as ps1,
            tc.tile_pool(name="ps2", bufs=2, space="PSUM") as ps2,
        ):
            # resident weights: w1 as 8x[128, F], w2 as 32x[128, C]
            w1s = [wpool.tile([128, F], bf16, tag=f"w1s{c}", name=f"w1s{c}") for c in range(CT)]
            w2s = [wpool.tile([128, C], bf16, tag=f"w2s{f}", name=f"w2s{f}") for f in range(FT)]
            for c in range(CT):
                nc.sync.dma_start(w1s[c][:], w1_d[c * 128:(c + 1) * 128, :])

            def load_w2():
                for f in range(FT):
                    nc.sync.dma_start(w2s[f][:], w2_d[f * 128:(f + 1) * 128, :])

            def body(emit_w2_after_first_xs: bool):
                first = True
                for (t0, ntok) in tok_tiles:
                    xs = [xpool.tile([128, ntok], bf16, tag="xs", name=f"xs{c}") for c in range(CT)]
                    for c in range(CT):
                        nc.sync.dma_start(xs[c][:], xT_d[c * 128:(c + 1) * 128, t0:t0 + ntok])
                    if first and emit_w2_after_first_xs:
                        # w2 queued behind w1 + first x tile: arrives well
                        # before the first mm2 needs it
                        load_w2()
                    first = False

                    # h[f] = gelu(w1[:, f].T @ x)   [128, ntok] per f-tile
                    hs = []
                    for f in range(FT):
                        acc = ps1.tile([128, ntok], f32, tag="ps1", name="acc1")
                        for c in range(CT):
                            nc.tensor.matmul(
                                acc[:],
                                w1s[c][:, f * 128:(f + 1) * 128],
                                xs[c][:],
                                start=(c == 0),
                                stop=(c == CT - 1),
                            )
                        h = hpool.tile([128, ntok], bf16, tag="hs", name="h")
                        nc.scalar.activation(h[:], acc[:], mybir.ActivationFunctionType.Gelu)
                        hs.append(h)

                    # y[c] = sum_f w2[f][:, c].T @ h[f]   [128, ntok] per c-tile
                    for c in range(CT):
                        acc = ps2.tile([128, ntok], f32, tag="ps2", name="acc2")
                        for f in range(FT):
                            nc.tensor.matmul(
                                acc[:],
                                w2s[f][:, c * 128:(c + 1) * 128],
                                hs[f][:],
                                start=(f == 0),
                                stop=(f == FT - 1),
                            )
                        y = ypool.tile([128, ntok], f32, tag="ys", name="y")
                        nc.vector.tensor_copy(y[:], acc[:])
                        nc.sync.dma_start(yT_d[c * 128:(c + 1) * 128, t0:t0 + ntok], y[:])

            if loop_iters > 1:
                load_w2()
                with tc.For_i(0, loop_iters, 1):
                    body(emit_w2_after_first_xs=False)
            else:
                body(emit_w2_after_first_xs=True)

    nc.compile()
    return nc


def _route(x2d: np.ndarray, gate_w: np.ndarray):
    """Host router in float64. Returns per-expert (token_idx, weight)."""
    logits = x2d.astype(np.float64) @ gate_w.astype(np.float64)
    logits -= logits.max(axis=-1, keepdims=True)
    p = np.exp(logits)
    p /= p.sum(axis=-1, keepdims=True)
    # top-2 experts (distinct), matching jax.lax.top_k ordering
    top1 = np.argmax(p, axis=-1)
    p_masked = p.copy()
    p_masked[np.arange(p.shape[0]), top1] = -1.0
    top2 = np.argmax(p_masked, axis=-1)
    w1 = p[np.arange(p.shape[0]), top1]
    w2 = p[np.arange(p.shape[0]), top2]
    s = w1 + w2
    w1, w2 = w1 / s, w2 / s
    topi = np.stack([top1, top2], axis=1)  # [N, 2]
    topw = np.stack([w1, w2], axis=1)      # [N, 2]
    per_expert = []
    for e in range(NUM_EXPERTS):
        sel = topi == e  # [N, 2]
        tok = np.nonzero(sel.any(axis=1))[0]
        w = topw[sel]  # row-major -> aligned with tok order
        per_expert.append((tok, w))
    return per_expert


def _make_in_maps(x2d, w1, w2, per_expert, cap):
    in_maps = []
    for e in range(NUM_EXPERTS):
        tok, _ = per_expert[e]
        xT = np.zeros((x2d.shape[1], cap), dtype=ml_dtypes.bfloat16)
        xT[:, : len(tok)] = x2d[tok].T.astype(ml_dtypes.bfloat16)
        in_maps.append(
            {
                "xT": xT,
                "w1": np.ascontiguousarray(w1[e]).astype(ml_dtypes.bfloat16),
                "w2": np.ascontiguousarray(w2[e]).astype(ml_dtypes.bfloat16),
            }
        )
    return in_maps


def kernel(x, gate_w, w1, w2):
    x = np.asarray(x, dtype=np.float32)
    gate_w = np.asarray(gate_w, dtype=np.float32)
    w1 = np.asarray(w1, dtype=np.float32)
    w2 = np.asarray(w2, dtype=np.float32)

    B, T, Cdim = x.shape
    N = B * T
    x2d = x.reshape(N, Cdim)

    per_expert = _route(x2d, gate_w)
    max_count = max(len(tok) for tok, _ in per_expert)
    cap = max(2304, -(-max_count // 128) * 128)

    key = ("ffn", cap)
    if key not in _cache:
        _cache[key] = _build(cap)
    nc = _cache[key]

    in_maps = _make_in_maps(x2d, w1, w2, per_expert, cap)
    res = run_bass_kernel_spmd(nc, in_maps, core_ids=list(range(N_CORES)))

    out = np.zeros((N, Cdim), dtype=np.float32)
    for e in range(NUM_EXPERTS):
        tok, wts = per_expert[e]
        yT = res.results[e]["yT"]  # [C, cap] f32
        out[tok] += wts[:, None].astype(np.float32) * yT[:, : len(tok)].T
    return out.reshape(B, T, Cdim)
